# revision 1
# baseline (speedup 1.0000x reference)
"""Additive (Bahdanau) attention on 8 TRN2 NeuronCores.

Math per batch b (masked positions contribute exactly 0 after exp):
    q = queries[b] @ W_q              (Q, H)
    k = keys[b]    @ W_k              (K, H)
    S[i, j] = sum_h w_v[h] * tanh(q[i,h] + k[j,h])
    out[b]  = softmax_j(S masked) @ values[b]

Sharding: the mask is a prefix mask (positions >= valid_len are dead), so
only sum(valid_lens) key columns carry work.  The host splits each batch's
valid-key prefix into jobs of KJ keys and hands each of the 8 cores NJ=2
jobs.  A job scores its KJ keys against all Q queries of its batch and
emits unnormalized partials (O^T = sum_j e^S v_j, l = sum_j e^S); the host
sums partials per batch and divides.  No max-subtraction is needed:
|S| <= sum|w_v| ~ 7, so exp never overflows in f32.

Device pipeline per key pair (2j, 2j+1), h in partitions:
  DVE  presum[:, :] = q2 + k2[:, j]      (tensor_scalar, per-partition addend)
  ACT  feats = tanh(presum)              (bulk over GP pairs, bf16 out)
  PE   S^T[2t:2t+2, :] += wpat_t^T @ feats   (zero-padded stationary lands
       each pair's scores at the right PSUM partitions -> S^T in key order)
  ACT  P = exp(S^T + mask_bias)          (prefix mask rides the bias)
  PE   O^T += V^T_block @ P ; l += 1^T @ P
"""

import sys

sys.path.insert(0, "/opt/trn_rl_repo")

import numpy as np

B, Q, KLEN, D_IN, H, D_V = 4, 1024, 1024, 256, 64, 128
NCORES = 8
NJ = 2  # jobs per core
MASK_VAL = -1.0e6
GP = 10  # key-pairs per bulk-tanh group

_CACHE = {}
LAST_RESULT = None


def _group_sizes(npairs, ramp_up, ramp_down):
    """Bulk-tanh group sizes: mostly GP, with small lead-in/lead-out groups at
    the kernel boundaries so ACT/PE pipeline fill+drain don't serialize (and
    the PE never idles past the HAM re-throttle window at the tail)."""
    up = [1, 1, 2, 4] if ramp_up else []
    down = [4, 2, 1, 1] if ramp_down else []
    mid = npairs - sum(up) - sum(down)
    if mid < 0:
        return [(2, "act")] * (npairs // 2) + [(1, "act")] * (npairs % 2)
    sizes = up + [GP] * (mid // GP) + ([mid % GP] if mid % GP else []) + down
    plan = []
    # 2-pair DVE-path (tanh addition formula) groups per block; fewer in the
    # final (ramp-down) block where the thinning pipeline can't hide them
    ndve = 1 if ramp_down else 5
    for s in sizes:
        if s == GP and ndve > 0:
            plan.append((s - 2, "act"))
            plan.append((2, "dve"))
            ndve -= 1
        else:
            plan.append((s, "act"))
    if ramp_up:
        # first two lead-in groups skip the DVE presum (bias-fused tanh):
        # shortens the kernel-start critical chain by the DVE hop
        plan[0] = (plan[0][0], "actb")
        plan[1] = (plan[1][0], "actb")
    assert sum(s for s, _ in plan) == npairs
    return plan


def _plan(vl):
    """Choose job size KJ and split batches' valid prefixes into NCORES*NJ jobs."""
    nslots = NCORES * NJ
    kj = 32
    while sum(-(-v // kj) for v in vl) > nslots:
        kj += 32
    jobs = []  # (batch, start, cnt)
    for b, v in enumerate(vl):
        nb_jobs = -(-v // kj)
        base, rem = divmod(v, nb_jobs)
        s = 0
        for i in range(nb_jobs):
            cnt = base + (1 if i < rem else 0)
            jobs.append((b, s, cnt))
            s += cnt
    while len(jobs) < nslots:
        jobs.append((0, 0, 0))  # empty padding job
    return kj, jobs


def _build(kj, repeat=1):
    import concourse.tile as tile
    from concourse import bacc, mybir

    fp32 = mybir.dt.float32
    bf16 = mybir.dt.bfloat16
    Tanh = mybir.ActivationFunctionType.Tanh
    Exp = mybir.ActivationFunctionType.Exp
    nbj = -(-kj // 128)  # key blocks per job
    hKJ = kj // 2

    nc = bacc.Bacc(
        "TRN2", target_bir_lowering=False, debug=False, num_devices=NCORES
    )
    qtsE = nc.dram_tensor("qts", [128, NJ * 2 * Q], bf16, kind="ExternalInput").ap()
    ktsE = nc.dram_tensor("kts", [128, NJ * 2 * kj], bf16, kind="ExternalInput").ap()
    vallE = nc.dram_tensor(
        "vall", [128, NJ * nbj * D_V], bf16, kind="ExternalInput"
    ).ap()
    mRE = nc.dram_tensor("maskR", [128, NJ * nbj], fp32, kind="ExternalInput").ap()
    wqE = nc.dram_tensor("wq", [128, 2 * H], bf16, kind="ExternalInput").ap()
    wkE = nc.dram_tensor("wk", [128, 2 * H], bf16, kind="ExternalInput").ap()
    wpE = nc.dram_tensor("wpat", [128, 512], bf16, kind="ExternalInput").ap()
    outE = nc.dram_tensor("out", [NJ * (D_V + 1), Q], fp32, kind="ExternalOutput").ap()

    with tile.TileContext(nc) as tc:
        with (
            tc.tile_pool(name="const", bufs=1) as cp,
            tc.tile_pool(name="feats", bufs=2) as fpool,
            tc.tile_pool(name="probs", bufs=2) as prp,
            tc.tile_pool(name="ps1", bufs=1, space="PSUM") as ps1,
            tc.tile_pool(name="ps2", bufs=2, space="PSUM") as ps2,
        ):
            # --- input DMAs: one contiguous transfer per tensor, spread over
            # both HWDGE rings (sync, scalar) + SWDGE (gpsimd); q-side first so
            # projections start early.
            qts = cp.tile([128, NJ * 2 * Q], bf16)
            for jn in range(NJ):
                nc.sync.dma_start(
                    qts[:, jn * 2 * Q : (jn + 1) * 2 * Q],
                    qtsE[:, jn * 2 * Q : (jn + 1) * 2 * Q],
                )
            wq_sb = cp.tile([128, 2 * H], bf16)
            nc.scalar.dma_start(wq_sb[:], wqE[:, :])
            wk_sb = cp.tile([128, 2 * H], bf16)
            nc.scalar.dma_start(wk_sb[:], wkE[:, :])
            kts = cp.tile([128, NJ * 2 * kj], bf16)
            nc.scalar.dma_start(kts[:], ktsE[:, :])
            wp_sb = cp.tile([128, 512], bf16)
            nc.gpsimd.dma_start(wp_sb[:], wpE[:, :])
            mask_sb = cp.tile([128, NJ * nbj], fp32)
            nc.gpsimd.dma_start(mask_sb[:], mRE[:, :])
            vall = cp.tile([128, NJ * nbj * D_V], bf16)
            nc.gpsimd.dma_start(vall[:], vallE[:, :])
            ones_sb = cp.tile([128, 1], bf16)
            nc.vector.memset(ones_sb[:], 1.0)

            o_sb = cp.tile([128, NJ * Q], fp32, name="o_sb")
            lo_sb = cp.tile([1, NJ * Q], fp32, name="lo_sb")

            def emit_proj(rep, jn):
                """Project one job's queries/keys; returns (q2, k2) SBUF tiles."""
                qof = jn * 2 * Q
                # q_proj^T: qh halves stacked in partitions of one bank
                qproj_ps = ps2.tile(
                    [128, 512], fp32, tag="S0", name=f"qproj_{rep}_{jn}", bufs=2
                )
                for qh in range(2):
                    for cc in range(2):
                        nc.tensor.matmul(
                            qproj_ps[64 * qh : 64 * qh + 64, :],
                            wq_sb[:, cc * H : (cc + 1) * H],
                            qts[:, qof + cc * Q + qh * 512 : qof + cc * Q + qh * 512 + 512],
                            start=(cc == 0),
                            stop=(cc == 1),
                        )
                q2_sb = cp.tile([128, Q], bf16, tag=f"q2_{jn}", name=f"q2_{rep}_{jn}")
                for qh in range(2):
                    if qh == 0 or jn > 0 or rep > 0:
                        nc.vector.tensor_copy(
                            q2_sb[0:64, qh * 512 : qh * 512 + 512],
                            qproj_ps[64 * qh : 64 * qh + 64, :],
                        )
                        nc.vector.tensor_copy(
                            q2_sb[64:128, qh * 512 : qh * 512 + 512],
                            qproj_ps[64 * qh : 64 * qh + 64, :],
                        )
                    else:
                        nc.scalar.copy(
                            q2_sb[0:64, qh * 512 : qh * 512 + 512],
                            qproj_ps[64 * qh : 64 * qh + 64, :],
                        )
                        nc.scalar.copy(
                            q2_sb[64:128, qh * 512 : qh * 512 + 512],
                            qproj_ps[64 * qh : 64 * qh + 64, :],
                        )
                # k2 = paired key projections: [:64] even keys, [64:] odd
                kof = jn * 2 * kj
                kproj_ps = ps2.tile(
                    [128, 512], fp32, tag="S1", name=f"kproj_{rep}_{jn}", bufs=2
                )
                for half in range(2):
                    for cc in range(2):
                        nc.tensor.matmul(
                            kproj_ps[64 * half : 64 * half + 64, 0:hKJ],
                            wk_sb[:, cc * H : (cc + 1) * H],
                            kts[:, kof + cc * kj + half * hKJ : kof + cc * kj + half * hKJ + hKJ],
                            start=(cc == 0),
                            stop=(cc == 1),
                        )
                k2_sb = cp.tile(
                    [128, hKJ], fp32, tag=f"k2_{jn}", name=f"k2_{rep}_{jn}"
                )
                nc.vector.tensor_copy(k2_sb[:], kproj_ps[:, 0:hKJ])
                return q2_sb, k2_sb

            for rep in range(repeat):
                # job 0's projections immediately; job 1's are emitted after
                # job 0's ramp-up groups (lower scheduler priority) so the
                # first tanh isn't stuck behind job 1's DVE copies.
                proj = {0: emit_proj(rep, 0)}

                for jn in range(NJ):
                    O_ps = [
                        ps1.tile(
                            [128, 512], fp32, tag=f"O{qh}", name=f"O{qh}_{rep}_{jn}"
                        )
                        for qh in range(2)
                    ]
                    l_ps = [
                        ps1.tile(
                            [1, 512], fp32, tag=f"l{qh}", name=f"l{qh}_{rep}_{jn}"
                        )
                        for qh in range(2)
                    ]
                    q2_sb, k2_sb = proj[jn]
                    tq2_sb = cp.tile(
                        [128, Q], fp32, tag=f"tq2_{jn}", name=f"tq2_{rep}_{jn}"
                    )
                    nc.scalar.activation(tq2_sb[:], q2_sb[:], Tanh)
                    tk2_sb = cp.tile(
                        [128, hKJ], fp32, tag=f"tk2_{jn}", name=f"tk2_{rep}_{jn}"
                    )
                    nc.scalar.activation(tk2_sb[:], k2_sb[:], Tanh)

                    # main loop over 128-key blocks of this job
                    for m in range(nbj):
                        kb = min(128, kj - m * 128)
                        npair_m = kb // 2
                        S_ps = [
                            ps2.tile(
                                [128, 512],
                                fp32,
                                tag=f"S{qh}",
                                name=f"S{qh}_{rep}_{jn}_{m}",
                                bufs=2,
                            )
                            for qh in range(2)
                        ]
                        gsizes = _group_sizes(
                            npair_m,
                            ramp_up=(rep == 0 and jn == 0 and m == 0),
                            ramp_down=(
                                rep == repeat - 1 and jn == NJ - 1 and m == nbj - 1
                            ),
                        )
                        tp0 = 0
                        for grp, (gp, eng) in enumerate(gsizes):
                            feats = fpool.tile(
                                [128, gp * Q],
                                bf16,
                                name=f"feats_{rep}_{jn}_{m}_{grp}",
                                tag="dfeats" if eng == "dve" else "feats",
                                bufs=2 if eng == "dve" else 3,
                            )
                            if eng == "dve":
                                # tanh(q+k) = (tq+tk)/(1+tq*tk), all on DVE
                                u_sb = fpool.tile(
                                    [128, gp * Q],
                                    fp32,
                                    name=f"u_{rep}_{jn}_{m}_{grp}",
                                    tag="upath",
                                    bufs=2,
                                )
                                for p in range(gp):
                                    j = 64 * m + tp0 + p
                                    nc.vector.tensor_scalar(
                                        u_sb[:, p * Q : (p + 1) * Q],
                                        tq2_sb[:],
                                        tk2_sb[:, j : j + 1],
                                        1.0,
                                        mybir.AluOpType.mult,
                                        mybir.AluOpType.add,
                                    )
                                nc.vector.reciprocal_approx_fast(
                                    u_sb[:, 0 : gp * Q], u_sb[:, 0 : gp * Q]
                                )
                                for p in range(gp):
                                    j = 64 * m + tp0 + p
                                    nc.vector.scalar_tensor_tensor(
                                        feats[:, p * Q : (p + 1) * Q],
                                        tq2_sb[:],
                                        tk2_sb[:, j : j + 1],
                                        u_sb[:, p * Q : (p + 1) * Q],
                                        mybir.AluOpType.add,
                                        mybir.AluOpType.mult,
                                    )
                            elif eng == "actb":
                                for p in range(gp):
                                    j = 64 * m + tp0 + p
                                    nc.scalar.activation(
                                        feats[:, p * Q : (p + 1) * Q],
                                        q2_sb[:],
                                        Tanh,
                                        bias=k2_sb[:, j : j + 1],
                                        scale=1.0,
                                    )
                            else:
                                presum = fpool.tile(
                                    [128, gp * Q],
                                    bf16,
                                    name=f"presum_{rep}_{jn}_{m}_{grp}",
                                    tag="presum",
                                    bufs=3,
                                )
                                for p in range(gp):
                                    j = 64 * m + tp0 + p
                                    nc.vector.tensor_scalar_add(
                                        presum[:, p * Q : (p + 1) * Q],
                                        q2_sb[:],
                                        k2_sb[:, j : j + 1],
                                    )
                                nc.scalar.activation(
                                    feats[:, 0 : gp * Q], presum[:, 0 : gp * Q], Tanh
                                )
                            for p in range(gp):
                                tp = tp0 + p
                                g, tl = divmod(tp, 16)
                                for qh in range(2):
                                    nc.tensor.matmul(
                                        S_ps[qh][32 * g : 32 * g + 32, :],
                                        wp_sb[:, 32 * tl : 32 * tl + 32],
                                        feats[:, p * Q + qh * 512 : p * Q + qh * 512 + 512],
                                        start=(tl == 0),
                                        stop=(tl == 15 or tp == npair_m - 1),
                                        tile_position=(0, 32 * g),
                                    )
                            tp0 += gp
                            if (
                                jn == 0
                                and m == 0
                                and grp == 9
                                and NJ > 1
                                and (jn + 1) not in proj
                            ):
                                proj[jn + 1] = emit_proj(rep, jn + 1)
                        if jn == 0 and m == 0 and NJ > 1 and 1 not in proj:
                            proj[1] = emit_proj(rep, 1)
                        for qh in range(2):
                            P_sb = prp.tile(
                                [128, 512],
                                bf16,
                                tag=f"P{qh}",
                                name=f"P{qh}_{rep}_{jn}_{m}",
                                bufs=2,
                            )
                            nc.scalar.activation(
                                P_sb[0:kb, :],
                                S_ps[qh][0:kb, :],
                                Exp,
                                bias=mask_sb[0:kb, jn * nbj + m : jn * nbj + m + 1],
                                scale=1.0,
                            )
                            nc.tensor.matmul(
                                O_ps[qh][:],
                                vall[0:kb, (jn * nbj + m) * D_V : (jn * nbj + m + 1) * D_V],
                                P_sb[0:kb, :],
                                start=(m == 0),
                                stop=(m == nbj - 1),
                            )
                            nc.tensor.matmul(
                                l_ps[qh][:],
                                ones_sb[0:kb, :],
                                P_sb[0:kb, :],
                                start=(m == 0),
                                stop=(m == nbj - 1),
                            )

                    for qh in range(2):
                        nc.vector.tensor_copy(
                            o_sb[:, jn * Q + qh * 512 : jn * Q + qh * 512 + 512],
                            O_ps[qh][:],
                        )
                        nc.vector.tensor_copy(
                            lo_sb[:, jn * Q + qh * 512 : jn * Q + qh * 512 + 512],
                            l_ps[qh][:],
                        )
                    if rep == repeat - 1:
                        nc.sync.dma_start(
                            outE[jn * (D_V + 1) : jn * (D_V + 1) + D_V, :],
                            o_sb[:, jn * Q : (jn + 1) * Q],
                        )
                        nc.sync.dma_start(
                            outE[jn * (D_V + 1) + D_V : jn * (D_V + 1) + D_V + 1, :],
                            lo_sb[:, jn * Q : (jn + 1) * Q],
                        )

    nc.compile()
    return nc


def _prepare(inputs):
    import ml_dtypes

    bf16 = ml_dtypes.bfloat16
    queries = np.asarray(inputs["queries"], dtype=np.float32)
    keys = np.asarray(inputs["keys"], dtype=np.float32)
    values = np.asarray(inputs["values"], dtype=np.float32)
    valid_lens = np.asarray(inputs["valid_lens"]).astype(np.int64)
    W_q = np.asarray(inputs["W_q"], dtype=np.float32)
    W_k = np.asarray(inputs["W_k"], dtype=np.float32)
    w_v = np.asarray(inputs["w_v"], dtype=np.float32)

    kj, jobs = _plan([int(x) for x in valid_lens])
    nbj = -(-kj // 128)

    wpat = np.zeros((128, 512), np.float32)
    for t in range(16):
        wpat[0:64, 32 * t + 2 * t] = w_v
        wpat[64:128, 32 * t + 2 * t + 1] = w_v
    wpat = wpat.astype(bf16)
    wq_r = np.concatenate([W_q[0:128], W_q[128:256]], axis=1).astype(bf16)
    wk_r = np.concatenate([W_k[0:128], W_k[128:256]], axis=1).astype(bf16)

    qT = {b: np.ascontiguousarray(queries[b].T) for b in range(B)}

    in_maps = []
    for c in range(NCORES):
        qts = np.empty((128, NJ * 2 * Q), bf16)
        kts = np.empty((128, NJ * 2 * kj), bf16)
        vall = np.zeros((128, NJ * nbj * D_V), bf16)
        maskR = np.full((128, NJ * nbj), MASK_VAL, np.float32)
        for jn in range(NJ):
            b, s, cnt = jobs[c * NJ + jn]
            qts[:, jn * 2 * Q : jn * 2 * Q + Q] = qT[b][0:128].astype(bf16)
            qts[:, jn * 2 * Q + Q : (jn + 1) * 2 * Q] = qT[b][128:256].astype(bf16)
            kp = np.zeros((kj, D_IN), np.float32)
            kp[0:cnt] = keys[b, s : s + cnt]
            kre = np.concatenate([kp[0::2], kp[1::2]], axis=0).T  # (256, kj)
            kts[:, jn * 2 * kj : jn * 2 * kj + kj] = kre[0:128].astype(bf16)
            kts[:, jn * 2 * kj + kj : (jn + 1) * 2 * kj] = kre[128:256].astype(bf16)
            vp = np.zeros((kj, D_V), np.float32)
            vp[0:cnt] = values[b, s : s + cnt]
            for m in range(nbj):
                kb = min(128, kj - m * 128)
                vall[0:kb, (jn * nbj + m) * D_V : (jn * nbj + m) * D_V + D_V] = vp[
                    m * 128 : m * 128 + kb
                ].astype(bf16)
                mm = np.full((128,), MASK_VAL, np.float32)
                nvalid = min(max(cnt - m * 128, 0), 128)
                mm[0:nvalid] = 0.0
                maskR[:, jn * nbj + m] = mm
        in_maps.append(
            {
                "qts": qts,
                "kts": kts,
                "vall": vall,
                "maskR": maskR,
                "wq": wq_r,
                "wk": wk_r,
                "wpat": wpat,
            }
        )
    return kj, jobs, in_maps


def kernel(**inputs):
    global LAST_RESULT
    kj, jobs, in_maps = _prepare(inputs)

    if kj not in _CACHE:
        _CACHE[kj] = _build(kj)
    nc = _CACHE[kj]

    from concourse.bass_utils import run_bass_kernel_spmd

    res = run_bass_kernel_spmd(nc, in_maps, core_ids=list(range(NCORES)))
    LAST_RESULT = res

    O = np.zeros((B, D_V, Q), np.float64)
    L = np.zeros((B, Q), np.float64)
    for c in range(NCORES):
        o = np.asarray(res.results[c]["out"])  # (NJ*(D_V+1), Q)
        for jn in range(NJ):
            b, s, cnt = jobs[c * NJ + jn]
            if cnt == 0:
                continue
            O[b] += o[jn * (D_V + 1) : jn * (D_V + 1) + D_V].astype(np.float64)
            L[b] += o[jn * (D_V + 1) + D_V].astype(np.float64)
    out = (O / L[:, None, :]).transpose(0, 2, 1)
    return np.ascontiguousarray(out.astype(np.float32))



# revision 10
# speedup vs baseline: 5.3488x; 5.3488x over previous
"""Additive (Bahdanau) attention on 8 TRN2 NeuronCores — separable-score
formulation.

Math per batch b:
    q = queries[b] @ W_q                  (Q, H)
    k = keys[b]    @ W_k                  (K, H)
    S[i, j] = sum_h w_v[h] * tanh(q[i,h] + k[j,h])
    out[b]  = softmax_j(S masked) @ values[b]

Instead of materializing the (Q, K, H) tanh tensor (elementwise-engine
bound), tanh is approximated by a regularized Fourier-extension series

    tanh(u) ~= c1*u + sum_{r=1..R} b_r sin(r*w1*u),   w1 = pi/PERIOD

whose angle-addition expansion is separable in (q, k):

    S[i,j] ~= sum_h sum_r (w_h b_r) [sin(r w1 q) cos(r w1 k)
                                     + cos(r w1 q) sin(r w1 k)]
              + c1 * sum_h w_h k_jh   (+ a per-row term softmax discards)

so S becomes R chained 128-contraction matmuls over trig features, and the
linear term folds into the exp bias.  The HW Sin table is only valid for
|arg| <= pi, so just sin/cos(w1 q) are computed on ACT (|w1 q| + pi/2 < pi)
and higher harmonics come from the Chebyshev angle ladder on DVE in fp16:

    X_r = 2 cos(w1 x) * X_{r-1} - X_{r-2}

which runs the sin and cos chains together in one (128, n) tile
(partitions = 64 h x {sin, cos}).  Rel err of the whole approximation
(incl. fp16 features) ~= 5e-3 vs the exact reference.

Sharding: the prefix mask means only sum(valid_lens) key columns carry
work.  Valid keys are split into <=128-key jobs; each of the 8 cores runs
4 job slots fed by two query feature sets (set0 serves slots 0-2, set1
serves slot 3; a set is one (batch, q-half) of 512 queries).  Each job
emits unnormalized partials (O^T = V^T P, l = 1^T P); the host sums
partials per batch and divides.
"""

import sys

sys.path.insert(0, "/opt/trn_rl_repo")

import numpy as np

B, Q, KLEN, D_IN, H, D_V = 4, 1024, 1024, 256, 64, 128
NCORES = 8
NSLOT = 4  # job slots per core: slots 0-2 -> set0, slot 3 -> set1
MASK_VAL = -1.0e6

R = 12  # Fourier harmonics
PERIOD = 13.0
OM1 = np.pi / PERIOD
UFIT = 9.8
LAM = 0.1

_CACHE = {}
_COEF = None
LAST_RESULT = None


def _fit_coeffs():
    """Host-side (weights-only) fit: tanh(u) ~= c1*u + sum b_r sin(r w1 u)."""
    global _COEF
    if _COEF is not None:
        return _COEF
    u = np.linspace(-UFIT, UFIT, 6001)
    w = 0.25 + np.exp(-(u**2) / (2 * 3.0**2))
    cols = [u / UFIT] + [np.sin((r + 1) * np.pi * u / PERIOD) for r in range(R)]
    A = np.stack(cols, axis=1)
    AtA = (A * w[:, None] ** 2).T @ A + LAM * np.eye(A.shape[1])
    Aty = (A * w[:, None] ** 2).T @ np.tanh(u)
    coef = np.linalg.solve(AtA, Aty)
    _COEF = (coef[0] / UFIT, coef[1:].copy())
    return _COEF


def _plan(vl):
    """Split each batch's valid-key prefix into <=128-key jobs per q-half and
    pack them into 8 cores x (one 3-job set0 + one 1-job set1).

    Returns cores: list of 8 entries, each a list of NSLOT jobs
    (b, qh, start, cnt) with cnt == 0 for padding slots."""
    combos = []  # ((b, qh), [(start, cnt), ...])
    for b, v in enumerate(vl):
        nb = -(-v // 128) if v > 0 else 0
        chunks = []
        s = 0
        for i in range(nb):
            cnt = min(128, v - s)
            chunks.append((s, cnt))
            s += cnt
        for qh in range(2):
            if chunks:
                combos.append(((b, qh), list(chunks)))
    assert sum(len(c[1]) for c in combos) <= 4 * NCORES, (
        "job count exceeds slot capacity; vl sum too large for this plan"
    )
    # Greedy: fill 8 triple-slots (set0) and 8 single-slots (set1).
    triples = []  # (combo, [jobs up to 3])
    singles = []  # (combo, job)
    combos = sorted(combos, key=lambda c: -len(c[1]))
    leftovers = []
    for key, chunks in combos:
        i = 0
        while len(chunks) - i >= 3 and len(triples) < NCORES:
            triples.append((key, chunks[i : i + 3]))
            i += 3
        leftovers.append((key, chunks[i:]))
    # place leftovers: prefer singles; group of 2-3 can take a triple slot
    for key, chunks in sorted(leftovers, key=lambda c: -len(c[1])):
        i = 0
        while len(chunks) - i >= 2 and len(triples) < NCORES:
            take = min(3, len(chunks) - i)
            triples.append((key, chunks[i : i + take]))
            i += take
        while i < len(chunks):
            if len(singles) < NCORES:
                singles.append((key, chunks[i]))
            elif len(triples) < NCORES:
                triples.append((key, [chunks[i]]))
            else:
                raise AssertionError("packing failed")
            i += 1
    while len(triples) < NCORES:
        triples.append(((0, 0), []))
    while len(singles) < NCORES:
        singles.append(((0, 0), None))
    cores = []
    for c in range(NCORES):
        (b0, qh0), t_jobs = triples[c]
        (b1, qh1), s_job = singles[c]
        jobs = []
        for i in range(3):
            if i < len(t_jobs):
                jobs.append((b0, qh0, t_jobs[i][0], t_jobs[i][1]))
            else:
                jobs.append((b0, qh0, 0, 0))
        if s_job is not None:
            jobs.append((b1, qh1, s_job[0], s_job[1]))
        else:
            jobs.append((b1, qh1, 0, 0))
        cores.append(jobs)
    return cores


def _build(repeat=1):
    import concourse.tile as tile
    from concourse import bacc, mybir

    fp32 = mybir.dt.float32
    fp16 = mybir.dt.float16
    Sin = mybir.ActivationFunctionType.Sin
    Exp = mybir.ActivationFunctionType.Exp
    Copy = mybir.ActivationFunctionType.Copy
    mult = mybir.AluOpType.mult
    sub = mybir.AluOpType.subtract
    add = mybir.AluOpType.add

    nc = bacc.Bacc(
        "TRN2", target_bir_lowering=False, debug=False, num_devices=NCORES
    )
    qtsE = nc.dram_tensor("qts", [128, 2048], fp16, kind="ExternalInput").ap()
    ktsE = nc.dram_tensor("kts", [128, 1024], fp16, kind="ExternalInput").ap()
    # vtsA: per job 65 cols = [V[:, 0:64] | ones]; the ones column makes the
    # O-matmul emit l (= 1^T P) as output row 64.  vtsB: V[:, 64:128].
    vtsAE = nc.dram_tensor("vtsA", [128, NSLOT * 65], fp16, kind="ExternalInput").ap()
    vtsBE = nc.dram_tensor("vtsB", [128, NSLOT * 64], fp16, kind="ExternalInput").ap()
    maskE = nc.dram_tensor("mask", [128, 4], fp32, kind="ExternalInput").ap()
    wq2E = nc.dram_tensor("wq2", [128, 256], fp16, kind="ExternalInput").ap()
    wk2E = nc.dram_tensor("wk2", [128, 256], fp16, kind="ExternalInput").ap()
    wvecE = nc.dram_tensor("wvec", [128, 1], fp16, kind="ExternalInput").ap()
    wbsE = nc.dram_tensor("wbs", [128, R], fp32, kind="ExternalInput").ap()
    hpE = nc.dram_tensor("hp", [128, 3], fp32, kind="ExternalInput").ap()
    outAE = nc.dram_tensor("outA", [65, NSLOT * 512], fp32, kind="ExternalOutput").ap()
    outBE = nc.dram_tensor("outB", [64, NSLOT * 512], fp32, kind="ExternalOutput").ap()

    with tile.TileContext(nc) as tc:
        with (
            tc.tile_pool(name="const", bufs=1) as cp,
            tc.tile_pool(name="feat", bufs=2) as fp,
            tc.tile_pool(name="ps", bufs=1, space="PSUM") as psp,
        ):
            # ---- input DMAs (q-path first), spread across rings
            qts = cp.tile([128, 2048], fp16)
            nc.sync.dma_start(qts[:, 0:1024], qtsE[:, 0:1024])
            wq2 = cp.tile([128, 256], fp16)
            nc.scalar.dma_start(wq2[:], wq2E[:, :])
            nc.sync.dma_start(qts[:, 1024:2048], qtsE[:, 1024:2048])
            kts = cp.tile([128, 1024], fp16)
            nc.scalar.dma_start(kts[:], ktsE[:, :])
            wk2 = cp.tile([128, 256], fp16)
            nc.scalar.dma_start(wk2[:], wk2E[:, :])
            hp = cp.tile([128, 3], fp32)
            nc.gpsimd.dma_start(hp[:], hpE[:, :])
            wbs = cp.tile([128, R], fp32)
            nc.gpsimd.dma_start(wbs[:], wbsE[:, :])
            vtsA = cp.tile([128, NSLOT * 65], fp16)
            nc.gpsimd.dma_start(vtsA[:], vtsAE[:, :])
            vtsB = cp.tile([128, NSLOT * 64], fp16)
            nc.gpsimd.dma_start(vtsB[:], vtsBE[:, :])
            mask_sb = cp.tile([128, 4], fp32)
            nc.gpsimd.dma_start(mask_sb[:], maskE[:, :])
            wvec = cp.tile([128, 1], fp16)
            nc.gpsimd.dma_start(wvec[:], wvecE[:, :])

            # ladder r=0 tiles: q-side [sin0; cos0] = [0; 1], k-side [cos0; sin0]
            x0q = cp.tile([128, 1024], fp16, name="x0q")
            nc.gpsimd.memset(x0q[0:64, :], 0.0)
            nc.gpsimd.memset(x0q[64:128, :], 1.0)
            x0k = cp.tile([128, 512], fp16, name="x0k")
            nc.gpsimd.memset(x0k[0:64, :], 1.0)
            nc.gpsimd.memset(x0k[64:128, :], 0.0)

            for rep in range(repeat):
                sfx = f"_{rep}"
                # ---- projections (stationaries duplicated so psum has both
                # halves: rows 0:64 and 64:128 hold identical h-values)
                qp_ps = [
                    psp.tile([128, 512], fp32, tag=f"qp{s}", name=f"qp{s}{sfx}")
                    for s in range(2)
                ]
                for s in range(2):
                    for c in range(2):
                        nc.tensor.matmul(
                            qp_ps[s][:],
                            wq2[:, c * 128 : (c + 1) * 128],
                            qts[:, s * 1024 + c * 512 : s * 1024 + (c + 1) * 512],
                            start=(c == 0),
                            stop=(c == 1),
                        )
                kp_ps = psp.tile([128, 512], fp32, tag="kp", name=f"kp{sfx}")
                for c in range(2):
                    nc.tensor.matmul(
                        kp_ps[:],
                        wk2[:, c * 128 : (c + 1) * 128],
                        kts[:, c * 512 : (c + 1) * 512],
                        start=(c == 0),
                        stop=(c == 1),
                    )

                # ---- ACT anchors (Sin table)
                xq = [fp.tile([128, 1024], fp16, tag="xq", name=f"xq1{sfx}", bufs=2)]
                cq1 = fp.tile([128, 1024], fp16, tag="cq1", name=f"cq1{sfx}", bufs=2)
                for s in range(2):
                    nc.scalar.activation(
                        xq[0][:, s * 512 : (s + 1) * 512],
                        qp_ps[s][:],
                        Sin,
                        bias=hp[:, 0:1],
                        scale=1.0,
                    )
                    nc.scalar.activation(
                        cq1[:, s * 512 : (s + 1) * 512],
                        qp_ps[s][:],
                        Sin,
                        bias=hp[:, 1:2],
                        scale=1.0,
                    )
                xk = [fp.tile([128, 512], fp16, tag="xk", name=f"xk1{sfx}", bufs=2)]
                nc.scalar.activation(
                    xk[0][:], kp_ps[:], Sin, bias=hp[:, 2:3], scale=1.0
                )
                ck1 = fp.tile([128, 512], fp16, tag="ck1", name=f"ck1{sfx}", bufs=2)
                nc.scalar.activation(ck1[:], kp_ps[:], Sin, bias=hp[:, 1:2], scale=1.0)
                kk = fp.tile([128, 512], fp16, tag="kk", name=f"kk{sfx}", bufs=2)
                nc.scalar.activation(kk[:], kp_ps[:], Copy, scale=1.0 / OM1)

                # doubled-cos multiplier tiles
                dq = fp.tile([128, 1024], fp16, tag="dq", name=f"dq{sfx}", bufs=2)
                nc.vector.tensor_scalar_mul(dq[:], cq1[:], 2.0)
                dk = fp.tile([128, 512], fp16, tag="dk", name=f"dk{sfx}", bufs=2)
                nc.vector.tensor_scalar_mul(dk[:], ck1[:], 2.0)

                # ---- scaled k-features for r=1 (ACT, Sin-table Copy)
                bk = [fp.tile([128, 512], fp16, tag="bk1", name=f"bk1{sfx}", bufs=2)]
                nc.scalar.activation(bk[0][:], xk[0][:], Copy, scale=wbs[:, 0:1])

                # ---- beta matmuls (linear-term bias per key)
                beta_ps = psp.tile([128, 4], fp32, tag="beta", name=f"beta{sfx}")
                for j in range(NSLOT):
                    nc.tensor.matmul(
                        beta_ps[:, j : j + 1],
                        kk[:, j * 128 : (j + 1) * 128],
                        wvec[:],
                        start=True,
                        stop=True,
                    )
                betam = fp.tile([128, 4], fp32, tag="betam", name=f"betam{sfx}", bufs=2)
                for j in range(NSLOT):
                    nc.vector.tensor_tensor(
                        betam[:, j : j + 1],
                        beta_ps[:, j : j + 1],
                        mask_sb[:, j : j + 1],
                        add,
                    )

                # ---- ladders (DVE) + scaled copies (ACT) + S passes (PE)
                # PSUM bank budget is 8: reuse the projection banks for the
                # first three S accumulators (their consumers all run early).
                stag = ("qp0", "qp1", "kp", "S3")
                S_ps = [
                    psp.tile([128, 512], fp32, tag=stag[j], name=f"S{j}{sfx}")
                    for j in range(NSLOT)
                ]
                setof = (0, 0, 0, 1)
                for j in range(NSLOT):
                    s = setof[j]
                    nc.tensor.matmul(
                        S_ps[j][:],
                        bk[0][:, j * 128 : (j + 1) * 128],
                        xq[0][:, s * 512 : (s + 1) * 512],
                        start=True,
                        stop=False,
                    )
                for r in range(2, R + 1):
                    # q-side ladder step
                    tq = fp.tile(
                        [128, 1024], fp16, tag="tq", name=f"tq{r}{sfx}", bufs=2
                    )
                    nc.vector.tensor_tensor(tq[:], dq[:], xq[-1][:], mult)
                    xq.append(
                        fp.tile([128, 1024], fp16, tag=f"xq{r % 3}", name=f"xq{r}{sfx}", bufs=2)
                    )
                    prev2q = x0q if r == 2 else xq[-3]
                    nc.vector.tensor_tensor(xq[-1][:], tq[:], prev2q[:], sub)
                    # k-side ladder step
                    tk = fp.tile(
                        [128, 512], fp16, tag="tk", name=f"tk{r}{sfx}", bufs=2
                    )
                    nc.vector.tensor_tensor(tk[:], dk[:], xk[-1][:], mult)
                    xk.append(
                        fp.tile([128, 512], fp16, tag=f"xk{r % 3}", name=f"xk{r}{sfx}", bufs=2)
                    )
                    prev2k = x0k if r == 2 else xk[-3]
                    nc.vector.tensor_tensor(xk[-1][:], tk[:], prev2k[:], sub)
                    # scaled k-features
                    bk.append(
                        fp.tile(
                            [128, 512], fp16, tag=f"bk{r % 2}", name=f"bk{r}{sfx}", bufs=2
                        )
                    )
                    nc.scalar.activation(
                        bk[-1][:], xk[-1][:], Copy, scale=wbs[:, r - 1 : r]
                    )
                    # S passes for all four jobs at this harmonic
                    for j in range(NSLOT):
                        s = setof[j]
                        nc.tensor.matmul(
                            S_ps[j][:],
                            bk[-1][:, j * 128 : (j + 1) * 128],
                            xq[-1][:, s * 512 : (s + 1) * 512],
                            start=False,
                            stop=(r == R),
                        )

                # ---- softmax numerator + weighted values (l rides as row 64
                # of the A-half O matmul via the ones column in vtsA)
                oa_sb = fp.tile(
                    [65, NSLOT * 512], fp32, tag="oasb", name=f"oa{sfx}", bufs=1
                )
                ob_sb = fp.tile(
                    [64, NSLOT * 512], fp32, tag="obsb", name=f"ob{sfx}", bufs=1
                )
                for j in range(NSLOT):
                    P_j = fp.tile(
                        [128, 512], fp16, tag=f"P{j % 2}", name=f"P{j}{sfx}", bufs=2
                    )
                    nc.scalar.activation(
                        P_j[:], S_ps[j][:], Exp, bias=betam[:, j : j + 1], scale=1.0
                    )
                    Oa_ps = psp.tile([65, 512], fp32, tag="Oa", name=f"Oa{j}{sfx}")
                    nc.tensor.matmul(
                        Oa_ps[:],
                        vtsA[:, j * 65 : (j + 1) * 65],
                        P_j[:],
                        start=True,
                        stop=True,
                    )
                    Ob_ps = psp.tile([64, 512], fp32, tag="Ob", name=f"Ob{j}{sfx}")
                    nc.tensor.matmul(
                        Ob_ps[:],
                        vtsB[:, j * 64 : (j + 1) * 64],
                        P_j[:],
                        start=True,
                        stop=True,
                    )
                    nc.vector.tensor_copy(oa_sb[:, j * 512 : (j + 1) * 512], Oa_ps[:])
                    nc.vector.tensor_copy(ob_sb[:, j * 512 : (j + 1) * 512], Ob_ps[:])
                    if rep == repeat - 1:
                        nc.sync.dma_start(
                            outAE[:, j * 512 : (j + 1) * 512],
                            oa_sb[:, j * 512 : (j + 1) * 512],
                        )
                        nc.sync.dma_start(
                            outBE[:, j * 512 : (j + 1) * 512],
                            ob_sb[:, j * 512 : (j + 1) * 512],
                        )

    nc.compile()
    return nc


def _prepare(inputs):
    import ml_dtypes

    f16 = np.float16
    queries = np.asarray(inputs["queries"], dtype=np.float32)
    keys = np.asarray(inputs["keys"], dtype=np.float32)
    values = np.asarray(inputs["values"], dtype=np.float32)
    valid_lens = np.asarray(inputs["valid_lens"]).astype(np.int64)
    W_q = np.asarray(inputs["W_q"], dtype=np.float32)
    W_k = np.asarray(inputs["W_k"], dtype=np.float32)
    w_v = np.asarray(inputs["w_v"], dtype=np.float32)

    c1lin, bco = _fit_coeffs()
    cores = _plan([int(x) for x in valid_lens])

    wq2 = np.empty((128, 256), f16)
    wk2 = np.empty((128, 256), f16)
    for c in range(2):
        wq2[:, c * 128 : (c + 1) * 128] = np.tile(
            W_q[c * 128 : (c + 1) * 128] * OM1, (1, 2)
        ).astype(f16)
        wk2[:, c * 128 : (c + 1) * 128] = np.tile(
            W_k[c * 128 : (c + 1) * 128] * OM1, (1, 2)
        ).astype(f16)
    wvec = np.tile((0.5 * c1lin * w_v)[:, None], (2, 1)).astype(f16)
    wbs = np.tile(w_v[:, None] * bco[None, :], (2, 1)).astype(np.float32)
    hp = np.zeros((128, 3), np.float32)
    hp[64:128, 0] = np.pi / 2
    hp[:, 1] = np.pi / 2
    hp[0:64, 2] = np.pi / 2

    qT = {b: np.ascontiguousarray(queries[b].T) for b in range(B)}

    in_maps = []
    for c in range(NCORES):
        jobs = cores[c]
        qts = np.zeros((128, 2048), f16)
        for s, j in ((0, 0), (1, 3)):
            b, qh, _, _ = jobs[j]
            for ch in range(2):
                qts[:, s * 1024 + ch * 512 : s * 1024 + (ch + 1) * 512] = qT[b][
                    ch * 128 : (ch + 1) * 128, qh * 512 : (qh + 1) * 512
                ].astype(f16)
        kts = np.zeros((128, 1024), f16)
        vtsA = np.zeros((128, NSLOT * 65), f16)
        vtsB = np.zeros((128, NSLOT * 64), f16)
        mask = np.full((128, 4), MASK_VAL, np.float32)
        for j, (b, qh, s0, cnt) in enumerate(jobs):
            vtsA[:, j * 65 + 64] = 1.0  # ones column -> l row
            if cnt == 0:
                continue
            kp = np.zeros((128, D_IN), np.float32)
            kp[0:cnt] = keys[b, s0 : s0 + cnt]
            kT = kp.T  # (256, 128)
            for ch in range(2):
                kts[:, ch * 512 + j * 128 : ch * 512 + (j + 1) * 128] = kT[
                    ch * 128 : (ch + 1) * 128
                ].astype(f16)
            vp = np.zeros((128, D_V), np.float32)
            vp[0:cnt] = values[b, s0 : s0 + cnt]
            vtsA[:, j * 65 : j * 65 + 64] = vp[:, 0:64].astype(f16)
            vtsB[:, j * 64 : (j + 1) * 64] = vp[:, 64:128].astype(f16)
            mask[0:cnt, j] = 0.0
        in_maps.append(
            {
                "qts": qts,
                "kts": kts,
                "vtsA": vtsA,
                "vtsB": vtsB,
                "mask": mask,
                "wq2": wq2,
                "wk2": wk2,
                "wvec": wvec,
                "wbs": wbs,
                "hp": hp,
            }
        )
    return cores, in_maps


def kernel(**inputs):
    global LAST_RESULT
    cores, in_maps = _prepare(inputs)

    if "nc" not in _CACHE:
        _CACHE["nc"] = _build()
    nc = _CACHE["nc"]

    from concourse.bass_utils import run_bass_kernel_spmd

    res = run_bass_kernel_spmd(nc, in_maps, core_ids=list(range(NCORES)))
    LAST_RESULT = res

    O = np.zeros((B, D_V, Q), np.float64)
    L = np.zeros((B, Q), np.float64)
    for c in range(NCORES):
        oA = np.asarray(res.results[c]["outA"]).astype(np.float64)
        oB = np.asarray(res.results[c]["outB"]).astype(np.float64)
        for j, (b, qh, s0, cnt) in enumerate(cores[c]):
            if cnt == 0:
                continue
            sl = slice(j * 512, (j + 1) * 512)
            qs = slice(qh * 512, (qh + 1) * 512)
            O[b][0:64, qs] += oA[0:64, sl]
            O[b][64:128, qs] += oB[:, sl]
            L[b][qs] += oA[64, sl]
    out = (O / L[:, None, :]).transpose(0, 2, 1)
    return np.ascontiguousarray(out.astype(np.float32))
